# revision 11
# baseline (speedup 1.0000x reference)
"""Bass/Tile kernel for nn_MultiHeadAttention (B=2, S=2048, D=1024, H=16) on 8 trn2 cores.

Sharding: core c -> (b = c//4, head-group hg = c%4). Each core computes 4 heads'
q/k/v projections, relu-attention, and a partial FC (256 of 1024 contraction rows).
Host pre-casts to bf16, pre-transposes x / weight slices, and sums the 4
partials per batch + bias.

v5 design (all-bf16 compute, fp32 PSUM accumulate):
  - every PSUM drain is FD=1024 so the DVE/ACT fixed cost amortizes:
    kq pairs (c=0,1 in one 2-bank tile), v quads (4 seq chunks / 2 banks),
    scores pairs (2 heads / 2 banks -> ONE 1024-wide relu per m-step),
    fc pairs (eb=0,1 / 2 banks -> one 1024-wide copyback + one wide y DMA)
  - scores/proj/fc share one 3-buf 2-bank PSUM pool: 3 scores slots in flight
    give the ~1.2us relu latency 3 m-steps of slack (the v4 2-slot rotation
    stalled scores(m+2) on relu(m) completion every step)
  - fc runs as PE-dense bursts at qb boundaries instead of inside the m-loop
  - qb0/hp0 attention interleaves into the projection phase quarter-by-quarter
    so relu work lands where DVE/ACT would otherwise idle
  - DMA: wk first on sync; x quarter-major k-pairs alternate sync/gpsimd;
    scalar/vector queues stay clean for relu/copyback work
  - y staged and stored as bf16 (halves output DMA bytes; host sums in f64)
  - PSUM budget (8 banks): pair pool 3x2, av 2x1
"""
import numpy as np
import ml_dtypes

import concourse.bass as bass
import concourse.mybir as mybir
import concourse.tile as tile

F32 = mybir.dt.float32
BF16 = mybir.dt.bfloat16
ts, ds = bass.ts, bass.ds

S = 2048
D = 1024
DL = 256      # per-core q/k/v dim (4 heads x 64)
P = 128
KD = D // P   # 8 k-chunks for projections
SQ = 512      # q-block (matmul N)
NQB = S // SQ # 4
NM = S // P   # 16 kpos chunks
DLC = DL // P # 2


def split_excess_waits(nc, max_embed: int = 1):
    """walrus core_v3 codegen accepts at most one sync-wait per instruction;
    move extra waits onto standalone event-sem instructions inserted before."""
    n_split = 0
    counter = 0
    for f in nc.m.functions:
        for blk in f.blocks:
            insts = blk.instructions
            if not any(
                ins.sync_info is not None and len(ins.sync_info.on_wait) > max_embed
                for ins in insts
            ):
                continue
            newl = []
            for ins in insts:
                si = ins.sync_info
                if si is not None and len(si.on_wait) > max_embed:
                    waits = list(si.on_wait)
                    extra, keep = waits[:-max_embed], waits[-max_embed:]
                    for w in extra:
                        counter += 1
                        es = mybir.InstEventSemaphore(name=f"waitsplit_{counter}")
                        es.engine = ins.engine
                        es.sync_info = mybir.SyncInfo(on_wait=[w], on_update=[])
                        newl.append(es)
                        n_split += 1
                    si.on_wait = keep
                newl.append(ins)
            blk.instructions = newl
    return n_split


def build_nc(with_mask: bool):
    nc = bass.Bass()
    # pre-arranged on host: x[p, c, s] = x.T[128c+p, s]; w[p, c, f] = w.T[128c+p, f]
    xT = nc.dram_tensor("xT", [P, KD, S], BF16, kind="ExternalInput")
    wq = nc.dram_tensor("wq", [P, KD, DL], BF16, kind="ExternalInput")
    wk = nc.dram_tensor("wk", [P, KD, DL], BF16, kind="ExternalInput")
    wv = nc.dram_tensor("wv", [P, KD, DL], BF16, kind="ExternalInput")
    wfc = nc.dram_tensor("wfc", [P, DLC, D], BF16, kind="ExternalInput")
    maskT = nc.dram_tensor("maskT", [S, S], F32, kind="ExternalInput") if with_mask else None
    y = nc.dram_tensor("y", [S, D], BF16, kind="ExternalOutput")

    with tile.TileContext(nc) as tc:
        _Emitter(tc, xT, wq, wk, wv, wfc, maskT, y).run()
    split_excess_waits(nc)
    return nc


class _Emitter:
    def __init__(self, tc, xT, wq, wk, wv, wfc, maskT, y):
        self.tc = tc
        self.nc = tc.nc
        self.xT, self.wq, self.wk, self.wv, self.wfc = xT, wq, wk, wv, wfc
        self.maskT, self.y = maskT, y
        self.cp = 0
        self.rl = 0
        self.dq = 0

    # -- engine alternation helpers ----------------------------------------
    def dma(self, out_ap, in_ap):
        eng = (self.nc.sync, self.nc.gpsimd)[self.dq % 2]
        eng.dma_start(out_ap, in_ap)
        self.dq += 1

    def copyback(self, out_ap, in_ap):
        if self.cp % 2 == 0:
            self.nc.vector.tensor_copy(out_ap, in_ap)
        else:
            self.nc.scalar.copy(out_ap, in_ap)
        self.cp += 1

    def relu(self, out_ap, in_ap):
        if self.rl % 2 == 0:
            self.nc.vector.tensor_scalar_max(out_ap, in_ap, 0.0)
        else:
            self.nc.scalar.activation(out_ap, in_ap, mybir.ActivationFunctionType.Relu)
        self.rl += 1

    # -- emission pieces ----------------------------------------------------
    def kq_pair(self, wsb, dstT, nb):
        """one projection group: dstT[:, 0:2, nb*SQ:...] via 16 matmuls into a
        2-bank pair tile, drained by one 1024-wide copyback. k-major order so
        the matmuls chase the arriving x k-pair DMA chunks; the two c-groups
        live in different banks so interleaving them is has_written-safe."""
        nc = self.nc
        pt = self.ps2.tile([P, DLC, SQ], F32, tag="p2", name=f"pj_{dstT.name}_{nb}")
        for k in range(KD):
            for c in range(DLC):
                nc.tensor.matmul(
                    pt[:, c, :], wsb[:, k, ts(c, P)], self.xb[:, k, ds(nb * SQ, SQ)],
                    start=(k == 0), stop=(k == KD - 1),
                )
        self.copyback(dstT[:, :, ds(nb * SQ, SQ)], pt[:, :, :])

    def v_quad(self, qv):
        """v projection for 4 seq chunks (one x quarter) into a 2-bank tile,
        one 1024-wide copyback into vN[:, 4qv:4qv+4, :]"""
        nc = self.nc
        pt = self.ps2.tile([P, 4, DL], F32, tag="p2", name=f"v_{qv}")
        # j-major: each j's accumulation group must fully finish before the
        # next group's start=True clears the bank's has_written bits (the
        # data survives, but an in-flight group would lose its partial sum)
        for j in range(4):
            for k in range(KD):
                nc.tensor.matmul(
                    pt[:, j, :], self.xb[:, k, ts(4 * qv + j, P)], self.wv_sb[:, k, :],
                    start=(k == 0), stop=(k == KD - 1),
                )
        self.copyback(self.vN[:, ds(4 * qv, 4), :], pt[:, :, :])

    def scores(self, qb, hp, m, attn_t, mtile):
        """both heads of pair hp into one 2-bank tile (concurrent row-tiled
        matmuls), then ONE 1024-wide relu into the bf16 attn tile"""
        nc = self.nc
        pt2 = self.ps2.tile([P, 2, SQ], F32, tag="p2", name=f"sc_{qb}_{hp}_{m}")
        for h in range(2):
            nc.tensor.matmul(
                pt2[:, h, :],
                self.kT[ds(64 * h, 64), hp, ts(m, P)],
                self.qT[ds(64 * h, 64), hp, ds(qb * SQ, SQ)],
                start=True, stop=True,
            )
        if mtile is not None:
            nc.vector.tensor_tensor(
                pt2[:, 0, :], pt2[:, 0, :], mtile[:, m, :], mybir.AluOpType.add
            )
            nc.vector.tensor_tensor(
                pt2[:, 1, :], pt2[:, 1, :], mtile[:, m, :], mybir.AluOpType.add
            )
        self.relu(attn_t[:, m, :, :], pt2[:, :, :])

    def av(self, qb, hp, m, attn_t, po):
        nc = self.nc
        for h in range(2):
            nc.tensor.matmul(
                po[ds(64 * h, 64), :],
                self.vN[:, m, ds(128 * hp + 64 * h, 64)],
                attn_t[:, m, h, :],
                start=(m == 0), stop=(m == NM - 1),
            )

    def attn_steps(self, qb, hp, at, po, mt, lo, hi):
        """emit steps [lo, hi) of the NM+3-step pipelined m-loop; av lags
        scores by 3 so a ~1.2us relu latency never gates the PE (3 scores
        PSUM slots -> relu(m-3) is the slot-release constraint anyway).
        A pending fc pair-group is injected after steps 7 and 15: the relu
        engines run ~60ns/step behind the PE, and the PE-dense fc lump gives
        them a catch-up window so the backlog never flushes at hp boundaries."""
        for m in range(lo, hi):
            if m < NM:
                self.scores(qb, hp, m, at, mt)
            if m >= 3:
                self.av(qb, hp, m - 3, at, po)
            if m in (7, 15) and self.fc_pending:
                self.fc_pair(self.fc_pending.pop(0))

    def fc_pair(self, sc):
        """fc for one 128-row output chunk: both eb halves accumulate in a
        2-bank tile (different banks -> interleave-safe), one 1024-wide
        copyback to a bf16 stage, one full-width y DMA"""
        nc = self.nc
        pt = self.ps2.tile([P, 2, SQ], F32, tag="p2", name=f"fc_{sc}")
        for eb in range(2):
            for c in range(DLC):
                nc.tensor.matmul(
                    pt[:, eb, :], self.outT[:, c, ts(sc, P)], self.wfc_sb[:, c, ds(eb * SQ, SQ)],
                    start=(c == 0), stop=(c == DLC - 1),
                )
        yt = self.ystage.tile([P, 2, SQ], BF16, tag="yt", name=f"yt_{sc}")
        self.copyback(yt[:, :, :], pt[:, :, :])
        self.dma(self.y[ts(sc, P), :], yt[:, :, :])

    def fc_single(self, sc, eb):
        """1-bank fc group through the av pool: smaller copyback latency and
        two parallel drain chains for the final burst's tail"""
        nc = self.nc
        pt = self.ps_av.tile([P, SQ], F32, tag="av", name=f"fcs_{sc}_{eb}")
        for c in range(DLC):
            nc.tensor.matmul(
                pt[:], self.outT[:, c, ts(sc, P)], self.wfc_sb[:, c, ds(eb * SQ, SQ)],
                start=(c == 0), stop=(c == DLC - 1),
            )
        yt = self.ystage.tile([P, 2, SQ], BF16, tag="yt", name=f"yts_{sc}_{eb}")
        self.copyback(yt[:, 0, :], pt[:])
        self.dma(self.y[ts(sc, P), ds(eb * SQ, SQ)], yt[:, 0, :])

    def load_mask(self, qb):
        if self.maskT is None:
            return None
        nc = self.nc
        mtile = self.mstg.tile([P, NM, SQ], F32, tag="mask", name=f"mask_{qb}")
        for m in range(NM):
            nc.sync.dma_start(
                mtile[:, m, :],
                self.maskT[:, :].rearrange("(m p) q -> p m q", p=P)[:, m, ds(qb * SQ, SQ)],
            )
        return mtile

    def attn_tile(self, qb, hp):
        return self.attn_pool.tile(
            [P, NM, 2, SQ], BF16, tag="attn", name=f"attn_{qb}_{hp}"
        )

    def av_tile(self, qb, hp):
        return self.ps_av.tile([P, SQ], F32, tag="av", name=f"av_{qb}_{hp}")

    # -- main ---------------------------------------------------------------
    def run(self):
        from contextlib import ExitStack

        tc, nc = self.tc, self.nc
        stack = ExitStack()
        sb = stack.enter_context(tc.tile_pool(name="sb", bufs=1))
        # PSUM budget (8 banks): 2-bank pair tiles x3 (proj/scores/fc), av 2x1
        self.ps2 = stack.enter_context(tc.tile_pool(name="ps2", bufs=3, space="PSUM"))
        self.ps_av = stack.enter_context(tc.tile_pool(name="ps_av", bufs=2, space="PSUM"))
        self.attn_pool = stack.enter_context(tc.tile_pool(name="attn", bufs=2))
        self.mstg = stack.enter_context(tc.tile_pool(name="mstg", bufs=2))
        self.ystage = stack.enter_context(tc.tile_pool(name="ystage", bufs=3))

        self.xb = sb.tile([P, KD, S], BF16, name="xb")
        self.wq_sb = sb.tile([P, KD, DL], BF16, name="wq_sb")
        self.wk_sb = sb.tile([P, KD, DL], BF16, name="wk_sb")
        self.wv_sb = sb.tile([P, KD, DL], BF16, name="wv_sb")
        self.wfc_sb = sb.tile([P, DLC, D], BF16, name="wfc_sb")
        self.qT = sb.tile([P, DLC, S], BF16, name="qT")
        self.kT = sb.tile([P, DLC, S], BF16, name="kT")
        self.vN = sb.tile([P, NM, DL], BF16, name="vN")
        self.outT = sb.tile([P, DLC, S], BF16, name="outT")

        self.fc_pending = []

        # loads: wk first half alone on sync (it gates the first projection
        # matmuls), first x quarter fanned over 4 queues (vector/scalar are
        # still idle this early), then quarter-major k-pairs on sync/gpsimd;
        # wv/wq early on gpsimd, wfc after quarter 2
        nc.sync.dma_start(self.wk_sb[:, 0:4, :], self.wk[:, 0:4, :])
        nc.gpsimd.dma_start(self.wk_sb[:, 4:8, :], self.wk[:, 4:8, :])
        # vector can't issue DMAs; scalar is still idle this early
        q0engs = (nc.sync, nc.scalar, nc.gpsimd, nc.scalar)
        for kp in range(KD // 2):
            q0engs[kp % 4].dma_start(
                self.xb[:, ds(2 * kp, 2), ds(0, SQ)],
                self.xT[:, ds(2 * kp, 2), ds(0, SQ)],
            )
        nc.gpsimd.dma_start(self.wv_sb[:], self.wv[:, :, :])
        nc.sync.dma_start(self.wq_sb[:], self.wq[:, :, :])
        for q in range(1, NQB):
            for kp in range(KD // 2):
                self.dma(
                    self.xb[:, ds(2 * kp, 2), ds(q * SQ, SQ)],
                    self.xT[:, ds(2 * kp, 2), ds(q * SQ, SQ)],
                )
            if q == 2:
                nc.gpsimd.dma_start(self.wfc_sb[:], self.wfc[:, :, :])

        # phase A: per x-quarter: k projection + v quad; q-projection for
        # qb0 after quarter 0; qb0/hp0 attention m-steps chase the kT/vN
        # quarters so relu work lands while DVE/ACT would otherwise idle
        mt0 = self.load_mask(0)
        at0 = self.attn_tile(0, 0)
        po0 = self.av_tile(0, 0)
        for q in range(NQB):
            self.kq_pair(self.wk_sb, self.kT, q)
            self.v_quad(q)
            if q == 0:
                self.kq_pair(self.wq_sb, self.qT, 0)
            else:
                self.attn_steps(0, 0, at0, po0, mt0, 4 * (q - 1), 4 * q)

        # finish qb0/hp0: m=12..15 plus the 3 lagging av steps
        self.attn_steps(0, 0, at0, po0, mt0, 12, NM + 3)
        self.copyback(self.outT[:, 0, ds(0, SQ)], po0[:])

        # phase B: remaining head-pair loops. Each qb's fc work is queued at
        # its end and injected into the NEXT qb's m-loops (PE-dense catch-up
        # lumps); q-projection for the next qb goes right after hp1 so its
        # copyback hides under trailing work. The last qb's fc runs as a
        # tail burst of 1-bank groups (shorter end drain).
        self.fc_pending = []
        for qb in range(NQB):
            mt = mt0 if qb == 0 else self.load_mask(qb)
            for hp in range(DLC):
                if qb == 0 and hp == 0:
                    continue  # already emitted in phase A
                at = self.attn_tile(qb, hp)
                po = self.av_tile(qb, hp)
                self.attn_steps(qb, hp, at, po, mt, 0, NM + 3)
                self.copyback(self.outT[:, hp, ds(qb * SQ, SQ)], po[:])
            if qb + 1 < NQB:
                self.kq_pair(self.wq_sb, self.qT, qb + 1)
                self.fc_pending += list(range(qb * 4, qb * 4 + 4))
            else:
                # drain any fc pairs not yet injected, then the final chunk
                while self.fc_pending:
                    self.fc_pair(self.fc_pending.pop(0))
                for sc in range(qb * 4, qb * 4 + 4):
                    for eb in range(2):
                        self.fc_single(sc, eb)

        stack.close()


# ---- host wrapper ---------------------------------------------------------

N_HEAD = 16
_nc_cache = {}


def get_nc(with_mask: bool):
    if with_mask not in _nc_cache:
        _nc_cache[with_mask] = build_nc(with_mask)
    return _nc_cache[with_mask]


def make_in_maps(x, mask, Wq, Wk, Wv, Wfc, with_mask):
    scale = np.float32(1.0 / np.sqrt(D // N_HEAD))
    bf = ml_dtypes.bfloat16
    in_maps = []
    for c in range(8):
        b, hg = divmod(c, 4)
        gs = slice(DL * hg, DL * hg + DL)
        def prearrange(wT, cdim):  # [cdim*128, F] -> [128, cdim, F]
            F = wT.shape[1]
            return np.ascontiguousarray(
                wT.reshape(cdim, P, F).transpose(1, 0, 2)
            ).astype(bf)

        m = {
            "xT": prearrange(x[b].T, KD),
            "wq": prearrange((Wq[gs, :] * scale).T, KD),
            "wk": prearrange(Wk[gs, :].T, KD),
            "wv": prearrange(Wv[gs, :].T, KD),
            "wfc": prearrange(Wfc[:, gs].T, DLC),
        }
        if with_mask:
            m["maskT"] = np.ascontiguousarray(
                np.broadcast_to(mask, (1, 1, S, S))[0, 0].T.astype(np.float32)
            )
        in_maps.append(m)
    return in_maps


def kernel(x, mask, Wq, Wk, Wv, Wfc, bfc):
    """Full-input entry: shards across 8 trn2 cores, returns the full output."""
    from concourse.bass_utils import run_bass_kernel_spmd

    x = np.asarray(x, dtype=np.float32)
    mask = np.asarray(mask, dtype=np.float32)
    Wq = np.asarray(Wq, dtype=np.float32)
    Wk = np.asarray(Wk, dtype=np.float32)
    Wv = np.asarray(Wv, dtype=np.float32)
    Wfc = np.asarray(Wfc, dtype=np.float32)
    bfc = np.asarray(bfc, dtype=np.float32)

    B = x.shape[0]
    with_mask = bool(np.any(mask))
    nc = get_nc(with_mask)
    in_maps = make_in_maps(x, mask, Wq, Wk, Wv, Wfc, with_mask)

    res = run_bass_kernel_spmd(nc, in_maps, core_ids=list(range(8)))
    parts = np.stack([np.asarray(r["y"], dtype=np.float64) for r in res.results])
    out = parts.reshape(B, 4, S, D).sum(axis=1)
    out += bfc.astype(np.float64)
    return out.astype(np.float32)


# revision 15
# speedup vs baseline: 1.0391x; 1.0391x over previous
"""Bass/Tile kernel for nn_MultiHeadAttention (B=2, S=2048, D=1024, H=16) on 8 trn2 cores.

Sharding: core c -> (b = c//4, head-group hg = c%4). Each core computes 4 heads'
q/k/v projections, relu-attention, and a partial FC (256 of 1024 contraction rows).
Host pre-casts to bf16, pre-transposes x / weight slices, and sums the 4
partials per batch + bias.

v5 design (all-bf16 compute, fp32 PSUM accumulate):
  - every PSUM drain is FD=1024 so the DVE/ACT fixed cost amortizes:
    kq pairs (c=0,1 in one 2-bank tile), v quads (4 seq chunks / 2 banks),
    scores pairs (2 heads / 2 banks -> ONE 1024-wide relu per m-step),
    fc pairs (eb=0,1 / 2 banks -> one 1024-wide copyback + one wide y DMA)
  - scores/proj/fc share one 3-buf 2-bank PSUM pool: 3 scores slots in flight
    give the ~1.2us relu latency 3 m-steps of slack (the v4 2-slot rotation
    stalled scores(m+2) on relu(m) completion every step)
  - fc runs as PE-dense bursts at qb boundaries instead of inside the m-loop
  - qb0/hp0 attention interleaves into the projection phase quarter-by-quarter
    so relu work lands where DVE/ACT would otherwise idle
  - DMA: wk first on sync; x quarter-major k-pairs alternate sync/gpsimd;
    scalar/vector queues stay clean for relu/copyback work
  - y staged and stored as bf16 (halves output DMA bytes; host sums in f64)
  - PSUM budget (8 banks): pair pool 3x2, av 2x1
"""
import numpy as np
import ml_dtypes

import concourse.bass as bass
import concourse.mybir as mybir
import concourse.tile as tile

F32 = mybir.dt.float32
BF16 = mybir.dt.bfloat16
ts, ds = bass.ts, bass.ds

S = 2048
D = 1024
DL = 256      # per-core q/k/v dim (4 heads x 64)
P = 128
KD = D // P   # 8 k-chunks for projections
SQ = 512      # q-block (matmul N)
NQB = S // SQ # 4
NM = S // P   # 16 kpos chunks
DLC = DL // P # 2


def split_excess_waits(nc, max_embed: int = 1):
    """walrus core_v3 codegen accepts at most one sync-wait per instruction;
    move extra waits onto standalone event-sem instructions inserted before."""
    n_split = 0
    counter = 0
    for f in nc.m.functions:
        for blk in f.blocks:
            insts = blk.instructions
            if not any(
                ins.sync_info is not None and len(ins.sync_info.on_wait) > max_embed
                for ins in insts
            ):
                continue
            newl = []
            for ins in insts:
                si = ins.sync_info
                if si is not None and len(si.on_wait) > max_embed:
                    waits = list(si.on_wait)
                    extra, keep = waits[:-max_embed], waits[-max_embed:]
                    for w in extra:
                        counter += 1
                        es = mybir.InstEventSemaphore(name=f"waitsplit_{counter}")
                        es.engine = ins.engine
                        es.sync_info = mybir.SyncInfo(on_wait=[w], on_update=[])
                        newl.append(es)
                        n_split += 1
                    si.on_wait = keep
                newl.append(ins)
            blk.instructions = newl
    return n_split


def build_nc(with_mask: bool):
    nc = bass.Bass()
    # pre-arranged on host: x[p, c, s] = x.T[128c+p, s]; w[p, c, f] = w.T[128c+p, f]
    xT = nc.dram_tensor("xT", [P, KD, S], BF16, kind="ExternalInput")
    wq = nc.dram_tensor("wq", [P, KD, DL], BF16, kind="ExternalInput")
    wk = nc.dram_tensor("wk", [P, KD, DL], BF16, kind="ExternalInput")
    wv = nc.dram_tensor("wv", [P, KD, DL], BF16, kind="ExternalInput")
    wfc = nc.dram_tensor("wfc", [P, DLC, D], BF16, kind="ExternalInput")
    maskT = nc.dram_tensor("maskT", [S, S], F32, kind="ExternalInput") if with_mask else None
    y = nc.dram_tensor("y", [S, D], BF16, kind="ExternalOutput")

    with tile.TileContext(nc) as tc:
        _Emitter(tc, xT, wq, wk, wv, wfc, maskT, y).run()
    split_excess_waits(nc)
    return nc


class _Emitter:
    def __init__(self, tc, xT, wq, wk, wv, wfc, maskT, y):
        self.tc = tc
        self.nc = tc.nc
        self.xT, self.wq, self.wk, self.wv, self.wfc = xT, wq, wk, wv, wfc
        self.maskT, self.y = maskT, y
        self.cp = 0
        self.rl = 0
        self.dq = 0

    # -- engine alternation helpers ----------------------------------------
    def dma(self, out_ap, in_ap):
        eng = (self.nc.sync, self.nc.gpsimd)[self.dq % 2]
        eng.dma_start(out_ap, in_ap)
        self.dq += 1

    def copyback(self, out_ap, in_ap):
        if self.cp % 2 == 0:
            self.nc.vector.tensor_copy(out_ap, in_ap)
        else:
            self.nc.scalar.copy(out_ap, in_ap)
        self.cp += 1

    def relu(self, out_ap, in_ap):
        if self.rl % 2 == 0:
            self.nc.vector.tensor_scalar_max(out_ap, in_ap, 0.0)
        else:
            self.nc.scalar.activation(out_ap, in_ap, mybir.ActivationFunctionType.Relu)
        self.rl += 1

    # -- emission pieces ----------------------------------------------------
    def kq_pair(self, wsb, dstT, nb):
        """one projection group: dstT[:, 0:2, nb*SQ:...] via 16 matmuls into a
        2-bank pair tile, drained by one 1024-wide copyback. k-major order so
        the matmuls chase the arriving x k-pair DMA chunks; the two c-groups
        live in different banks so interleaving them is has_written-safe."""
        nc = self.nc
        pt = self.ps2.tile([P, DLC, SQ], F32, tag="p2", name=f"pj_{dstT.name}_{nb}")
        for k in range(KD):
            for c in range(DLC):
                nc.tensor.matmul(
                    pt[:, c, :], wsb[:, k, ts(c, P)], self.xb[:, k, ds(nb * SQ, SQ)],
                    start=(k == 0), stop=(k == KD - 1),
                )
        self.copyback(dstT[:, :, ds(nb * SQ, SQ)], pt[:, :, :])

    def v_quad(self, qv):
        """v projection for 4 seq chunks (one x quarter) into a 2-bank tile,
        one 1024-wide copyback into vN[:, 4qv:4qv+4, :]"""
        nc = self.nc
        pt = self.ps2.tile([P, 4, DL], F32, tag="p2", name=f"v_{qv}")
        # j-major: each j's accumulation group must fully finish before the
        # next group's start=True clears the bank's has_written bits (the
        # data survives, but an in-flight group would lose its partial sum)
        for j in range(4):
            for k in range(KD):
                nc.tensor.matmul(
                    pt[:, j, :], self.xb[:, k, ts(4 * qv + j, P)], self.wv_sb[:, k, :],
                    start=(k == 0), stop=(k == KD - 1),
                )
        self.copyback(self.vN[:, ds(4 * qv, 4), :], pt[:, :, :])

    def scores(self, qb, hp, m, attn_t, mtile):
        """both heads of pair hp into one 2-bank tile (concurrent row-tiled
        matmuls), then ONE 1024-wide relu into the bf16 attn tile"""
        nc = self.nc
        pt2 = self.ps2.tile([P, 2, SQ], F32, tag="p2", name=f"sc_{qb}_{hp}_{m}")
        for h in range(2):
            nc.tensor.matmul(
                pt2[:, h, :],
                self.kT[ds(64 * h, 64), hp, ts(m, P)],
                self.qT[ds(64 * h, 64), hp, ds(qb * SQ, SQ)],
                start=True, stop=True,
            )
        if mtile is not None:
            nc.vector.tensor_tensor(
                pt2[:, 0, :], pt2[:, 0, :], mtile[:, m, :], mybir.AluOpType.add
            )
            nc.vector.tensor_tensor(
                pt2[:, 1, :], pt2[:, 1, :], mtile[:, m, :], mybir.AluOpType.add
            )
        self.relu(attn_t[:, m, :, :], pt2[:, :, :])

    def av(self, qb, hp, m, attn_t, po):
        nc = self.nc
        for h in range(2):
            nc.tensor.matmul(
                po[ds(64 * h, 64), :],
                self.vN[:, m, ds(128 * hp + 64 * h, 64)],
                attn_t[:, m, h, :],
                start=(m == 0), stop=(m == NM - 1),
            )

    def attn_qb(self, qb, mt, lo, hi):
        """fused virtual loop over BOTH head pairs of a q-block: steps
        0..2*NM+2, scores at s (hp = s//NM), av lagging 3 steps. Fusing the
        two hp loops means hp0's trailing relus drain under hp1's first
        scores instead of flushing the pipeline. av lag 3 matches the 3
        scores PSUM slots (relu(s-3) is the slot-release constraint).
        A pending fc pair-group is injected at s=7 and s=23: the relu
        engines run ~60ns/step behind the PE and the PE-dense fc lump gives
        them a catch-up window mid-loop."""
        for s in range(lo, hi):
            if s < 2 * NM:
                hp, mm = divmod(s, NM)
                if mm == 0:
                    self.at[hp] = self.attn_tile(qb, hp)
                    self.po[hp] = self.av_tile(qb, hp)
                self.scores(qb, hp, mm, self.at[hp], mt)
            if s >= 3:
                hp, mm = divmod(s - 3, NM)
                self.av(qb, hp, mm, self.at[hp], self.po[hp])
                if mm == NM - 1:
                    self.copyback(self.outT[:, hp, ds(qb * SQ, SQ)], self.po[hp][:])
            if s in (7, 23) and self.fc_pending:
                self.fc_pair(self.fc_pending.pop(0))

    def fc_pair(self, sc):
        """fc for one 128-row output chunk: both eb halves accumulate in a
        2-bank tile (different banks -> interleave-safe), one 1024-wide
        copyback to a bf16 stage, one full-width y DMA"""
        nc = self.nc
        pt = self.ps2.tile([P, 2, SQ], F32, tag="p2", name=f"fc_{sc}")
        for eb in range(2):
            for c in range(DLC):
                nc.tensor.matmul(
                    pt[:, eb, :], self.outT[:, c, ts(sc, P)], self.wfc_sb[:, c, ds(eb * SQ, SQ)],
                    start=(c == 0), stop=(c == DLC - 1),
                )
        yt = self.ystage.tile([P, 2, SQ], BF16, tag="yt", name=f"yt_{sc}")
        self.copyback(yt[:, :, :], pt[:, :, :])
        self.dma(self.y[ts(sc, P), :], yt[:, :, :])

    def fc_single(self, sc, eb):
        """1-bank fc group through the av pool: smaller copyback latency and
        two parallel drain chains for the final burst's tail"""
        nc = self.nc
        pt = self.ps_av.tile([P, SQ], F32, tag="av", name=f"fcs_{sc}_{eb}")
        for c in range(DLC):
            nc.tensor.matmul(
                pt[:], self.outT[:, c, ts(sc, P)], self.wfc_sb[:, c, ds(eb * SQ, SQ)],
                start=(c == 0), stop=(c == DLC - 1),
            )
        yt = self.ystage.tile([P, 2, SQ], BF16, tag="yt", name=f"yts_{sc}_{eb}")
        self.copyback(yt[:, 0, :], pt[:])
        self.dma(self.y[ts(sc, P), ds(eb * SQ, SQ)], yt[:, 0, :])

    def load_mask(self, qb):
        if self.maskT is None:
            return None
        nc = self.nc
        mtile = self.mstg.tile([P, NM, SQ], F32, tag="mask", name=f"mask_{qb}")
        for m in range(NM):
            nc.sync.dma_start(
                mtile[:, m, :],
                self.maskT[:, :].rearrange("(m p) q -> p m q", p=P)[:, m, ds(qb * SQ, SQ)],
            )
        return mtile

    def attn_tile(self, qb, hp):
        return self.attn_pool.tile(
            [P, NM, 2, SQ], BF16, tag="attn", name=f"attn_{qb}_{hp}"
        )

    def av_tile(self, qb, hp):
        return self.ps_av.tile([P, SQ], F32, tag="av", name=f"av_{qb}_{hp}")

    # -- main ---------------------------------------------------------------
    def run(self):
        from contextlib import ExitStack

        tc, nc = self.tc, self.nc
        stack = ExitStack()
        sb = stack.enter_context(tc.tile_pool(name="sb", bufs=1))
        # PSUM budget (8 banks): 2-bank pair tiles x3 (proj/scores/fc), av 2x1
        self.ps2 = stack.enter_context(tc.tile_pool(name="ps2", bufs=3, space="PSUM"))
        self.ps_av = stack.enter_context(tc.tile_pool(name="ps_av", bufs=2, space="PSUM"))
        self.attn_pool = stack.enter_context(tc.tile_pool(name="attn", bufs=2))
        self.mstg = stack.enter_context(tc.tile_pool(name="mstg", bufs=2))
        self.ystage = stack.enter_context(tc.tile_pool(name="ystage", bufs=3))

        self.xb = sb.tile([P, KD, S], BF16, name="xb")
        self.wq_sb = sb.tile([P, KD, DL], BF16, name="wq_sb")
        self.wk_sb = sb.tile([P, KD, DL], BF16, name="wk_sb")
        self.wv_sb = sb.tile([P, KD, DL], BF16, name="wv_sb")
        self.wfc_sb = sb.tile([P, DLC, D], BF16, name="wfc_sb")
        self.qT = sb.tile([P, DLC, S], BF16, name="qT")
        self.kT = sb.tile([P, DLC, S], BF16, name="kT")
        self.vN = sb.tile([P, NM, DL], BF16, name="vN")
        self.outT = sb.tile([P, DLC, S], BF16, name="outT")

        self.fc_pending = []
        self.at = {}
        self.po = {}

        # loads on two queues, ordered by first-use: wk half-0 and x kp0
        # land in parallel (nothing else competing) so the first projection
        # matmuls start ~10us in; remaining quarter-0 chunks + wk half-1
        # chase the accumulation loop, wv before wq (v_quad runs first)
        def dma_x(q, kp, eng):
            eng.dma_start(
                self.xb[:, ds(2 * kp, 2), ds(q * SQ, SQ)],
                self.xT[:, ds(2 * kp, 2), ds(q * SQ, SQ)],
            )
        nc.sync.dma_start(self.wk_sb[:, 0:4, :], self.wk[:, 0:4, :])
        dma_x(0, 0, nc.gpsimd)
        dma_x(0, 1, nc.sync)
        nc.gpsimd.dma_start(self.wk_sb[:, 4:8, :], self.wk[:, 4:8, :])
        dma_x(0, 3, nc.sync)
        dma_x(0, 2, nc.gpsimd)
        nc.sync.dma_start(self.wq_sb[:], self.wq[:, :, :])
        nc.gpsimd.dma_start(self.wv_sb[:], self.wv[:, :, :])
        for q in range(1, NQB):
            for kp in range(KD // 2):
                self.dma(
                    self.xb[:, ds(2 * kp, 2), ds(q * SQ, SQ)],
                    self.xT[:, ds(2 * kp, 2), ds(q * SQ, SQ)],
                )
            if q == 2:
                nc.gpsimd.dma_start(self.wfc_sb[:], self.wfc[:, :, :])

        # phase A: per x-quarter: v quad + k projection; q-projection for
        # qb0 after quarter 0; qb0/hp0 attention steps chase the kT/vN
        # quarters so relu work lands while DVE/ACT would otherwise idle
        mt0 = self.load_mask(0)
        for q in range(NQB):
            self.kq_pair(self.wk_sb, self.kT, q)
            self.v_quad(q)
            if q == 0:
                self.kq_pair(self.wq_sb, self.qT, 0)
            else:
                self.attn_qb(0, mt0, 4 * (q - 1), 4 * q)

        # finish qb0: hp0 m=12..15, all of hp1, trailing avs
        self.attn_qb(0, mt0, 12, 2 * NM + 3)

        # phase B boundaries: after each qb, queue its fc work, emit the
        # next qb's q-projection, then two fc pairs to cover the qT
        # copyback latency; the other two pairs inject into the next qb's
        # fused loop. The last qb's fc runs as 1-bank groups (short drain).
        for qb in range(NQB):
            if qb > 0:
                self.attn_qb(qb, self.load_mask(qb), 0, 2 * NM + 3)
            if qb + 1 < NQB:
                self.fc_pending += list(range(qb * 4, qb * 4 + 4))
                self.kq_pair(self.wq_sb, self.qT, qb + 1)
                self.fc_pair(self.fc_pending.pop(0))
                self.fc_pair(self.fc_pending.pop(0))
            else:
                while self.fc_pending:
                    self.fc_pair(self.fc_pending.pop(0))
                for sc in range(qb * 4, qb * 4 + 4):
                    for eb in range(2):
                        self.fc_single(sc, eb)

        stack.close()


# ---- host wrapper ---------------------------------------------------------

N_HEAD = 16
_nc_cache = {}


def get_nc(with_mask: bool):
    if with_mask not in _nc_cache:
        _nc_cache[with_mask] = build_nc(with_mask)
    return _nc_cache[with_mask]


def make_in_maps(x, mask, Wq, Wk, Wv, Wfc, with_mask):
    scale = np.float32(1.0 / np.sqrt(D // N_HEAD))
    bf = ml_dtypes.bfloat16
    in_maps = []
    for c in range(8):
        b, hg = divmod(c, 4)
        gs = slice(DL * hg, DL * hg + DL)
        def prearrange(wT, cdim):  # [cdim*128, F] -> [128, cdim, F]
            F = wT.shape[1]
            return np.ascontiguousarray(
                wT.reshape(cdim, P, F).transpose(1, 0, 2)
            ).astype(bf)

        m = {
            "xT": prearrange(x[b].T, KD),
            "wq": prearrange((Wq[gs, :] * scale).T, KD),
            "wk": prearrange(Wk[gs, :].T, KD),
            "wv": prearrange(Wv[gs, :].T, KD),
            "wfc": prearrange(Wfc[:, gs].T, DLC),
        }
        if with_mask:
            m["maskT"] = np.ascontiguousarray(
                np.broadcast_to(mask, (1, 1, S, S))[0, 0].T.astype(np.float32)
            )
        in_maps.append(m)
    return in_maps


def kernel(x, mask, Wq, Wk, Wv, Wfc, bfc):
    """Full-input entry: shards across 8 trn2 cores, returns the full output."""
    from concourse.bass_utils import run_bass_kernel_spmd

    x = np.asarray(x, dtype=np.float32)
    mask = np.asarray(mask, dtype=np.float32)
    Wq = np.asarray(Wq, dtype=np.float32)
    Wk = np.asarray(Wk, dtype=np.float32)
    Wv = np.asarray(Wv, dtype=np.float32)
    Wfc = np.asarray(Wfc, dtype=np.float32)
    bfc = np.asarray(bfc, dtype=np.float32)

    B = x.shape[0]
    with_mask = bool(np.any(mask))
    nc = get_nc(with_mask)
    in_maps = make_in_maps(x, mask, Wq, Wk, Wv, Wfc, with_mask)

    res = run_bass_kernel_spmd(nc, in_maps, core_ids=list(range(8)))
    parts = np.stack([np.asarray(r["y"], dtype=np.float64) for r in res.results])
    out = parts.reshape(B, 4, S, D).sum(axis=1)
    out += bfc.astype(np.float64)
    return out.astype(np.float32)
